# revision 1
# baseline (speedup 1.0000x reference)
"""Bass/Trainium2 kernel for nn_BasicBlock_73933567033945 (CDConv / gnn_message_passing).

Strategy: graph is a fixed +-8 sequence window inside each of 4 chains of
L=2048 nodes (verified against the src/dst inputs at runtime). Shard the
8192 nodes across 8 NeuronCores (1024 nodes each, half a chain) with an
8-node halo; all gathers become partition shifts materialized by PE
shift-matmuls, the per-edge kernel is a PE matmul against a block-diagonal
weight, the kern (x) h bilinear runs as 17 broadcast multiplies on DVE, and
the (offset, channel) contraction runs on the PE via PSUM-accumulated
transposes followed by Wk-chunk matmuls. Pure data parallel: no collectives.
"""
import numpy as np

B, L, C = 4, 2048, 128
N = B * L
W = 32
KC = 24
SEQ_L = 11
R = 12.0
WIN = 8
NEG_IN = 0.1
NEG_K = 0.2
NCORES = 8
NPC = N // NCORES          # 1024 nodes per core
TS = 112                   # output nodes per tile
NT = 10                    # tiles per core (9*112 + 16)
HR = 9 * TS + 128          # 1136 halo rows per core
K17 = 2 * WIN + 1          # 17 window offsets
S_HALF = SEQ_L // 2

_PROG = {}


def _sidx(k):
    return int(np.clip(k - WIN, -S_HALF, S_HALF)) + S_HALF


def _build_program():
    import concourse.tile as tile
    from concourse import mybir, bacc
    from concourse.bass_utils import run_bass_kernel_spmd  # noqa: F401 (import check)
    from contextlib import ExitStack

    f32 = mybir.dt.float32
    AF = mybir.ActivationFunctionType
    OP = mybir.AluOpType
    AX = mybir.AxisListType

    nc = bacc.Bacc("TRN2", target_bir_lowering=False, debug=False)

    def din(name, shape):
        return nc.dram_tensor(name, shape, f32, kind="ExternalInput").ap()

    x_slot = din("x_slot", [128, NT * C])
    xc_slot = din("xc_slot", [128, NT * C])
    po_slot = din("po_slot", [128, NT * 12])
    w_in = din("w_in", [C, W])
    ws_a = din("ws_a", [128, K17 * KC])
    ws_b = din("ws_b", [8, K17 * KC])
    wk_p = din("wk_p", [128, 6 * W])
    w_out = din("w_out", [W, C])
    ident = din("ident", [128, 128])
    shifts = din("shifts", [128, K17 * TS])
    w5r = din("w5r", [128, 3 * KC])
    b5r = din("b5r", [128, KC])
    maskd = din("maskd", [128, NT * K17])
    nclmp = din("nclmp", [128, NT])
    alph1 = din("alph1", [128, 1])
    alph2 = din("alph2", [128, 1])
    y = nc.dram_tensor("y", [NPC, C], f32, kind="ExternalOutput").ap()

    with tile.TileContext(nc) as tc, ExitStack() as ctx:
        pers = ctx.enter_context(tc.tile_pool(name="pers", bufs=1))

        def load(ap_in, shape, tag):
            t = pers.tile(shape, f32, tag=tag)
            nc.sync.dma_start(t[:], ap_in)
            return t

        x_all = load(x_slot, [128, NT * C], "x_all")
        xc_all = load(xc_slot, [128, NT * C], "xc_all")
        # phys: per slot j, 44 cols [h(32) | pos(3) | ori(9)]
        phys = pers.tile([128, NT * 44], f32, tag="phys")
        nc.sync.dma_start(
            phys[:].rearrange("p (j c) -> p j c", c=44)[:, :, 32:44],
            po_slot.rearrange("p (j c) -> p j c", c=12))
        w_in_sb = load(w_in, [C, W], "w_in")
        ws_a_sb = load(ws_a, [128, K17 * KC], "ws_a")
        ws_b_sb = load(ws_b, [8, K17 * KC], "ws_b")
        wk_sb = load(wk_p, [128, 6 * W], "wk")
        w_out_sb = load(w_out, [W, C], "w_out")
        id_sb = load(ident, [128, 128], "ident")
        sh_sb = load(shifts, [128, K17 * TS], "shifts")
        w5r_sb = load(w5r, [128, 3 * KC], "w5r")
        b5r_sb = load(b5r, [128, KC], "b5r")
        mask_sb = load(maskd, [128, NT * K17], "mask")
        ncl_sb = load(nclmp, [128, NT], "nclmp")
        a1_sb = load(alph1, [128, 1], "a1")
        a2_sb = load(alph2, [128, 1], "a2")
        bf16 = mybir.dt.bfloat16
        idb_sb = pers.tile([128, 128], bf16, tag="identb")
        nc.vector.tensor_copy(idb_sb[:], id_sb[:])


        # ---------------- Phase A: h = lrelu(lrelu(x) @ W_in) per slot -----
        with tc.tile_pool(name="pA", bufs=2) as pA, \
             tc.tile_pool(name="pAp", bufs=2, space="PSUM") as pAp:
            for j in range(NT):
                xl = pA.tile([128, C], f32, tag="xl")
                nc.scalar.activation(xl[:], x_all[:, j * C:(j + 1) * C],
                                     AF.Prelu, bias=0.0, scale=1.0,
                                     alpha=a1_sb[:, 0:1])
                xT_p = pAp.tile([128, 128], f32, tag="xT")
                nc.tensor.matmul(xT_p[:], xl[:], id_sb[:], is_transpose=True,
                                 start=True, stop=True)
                xT = pA.tile([128, 128], f32, tag="xTsb")
                nc.scalar.copy(xT[:], xT_p[:])
                hp = pAp.tile([128, W], f32, tag="hp")
                nc.tensor.matmul(hp[:], xT[:], w_in_sb[:], start=True, stop=True)
                nc.scalar.activation(phys[:, 44 * j:44 * j + W], hp[:],
                                     AF.Prelu, bias=0.0, scale=1.0,
                                     alpha=a1_sb[:, 0:1])

        # ---------------- Phase B: per output tile ------------------------
        wrk = ctx.enter_context(tc.tile_pool(name="wrk", bufs=2))
        tpool = ctx.enter_context(tc.tile_pool(name="tmp", bufs=4))
        ps = ctx.enter_context(tc.tile_pool(name="ps", bufs=1, space="PSUM"))
        ps2 = ctx.enter_context(tc.tile_pool(name="ps2", bufs=2, space="PSUM"))

        P = TS  # 112 active partitions
        for t in range(NT):
            # ---- neighborhood materialization via shift matmuls ----------
            # nb layout: k-block (44 cols = h|pos|ori) at col(k); k<=10 in
            # bank0 (44k), k>=11 in bank1 (512+44(k-11)) so no matmul output
            # crosses a PSUM bank boundary.
            def ncol(k):
                return 44 * k if k <= 10 else 512 + 44 * (k - 11)
            nb_p = ps.tile([P, 820], f32, tag="nb")
            for k in range(K17):
                nc.tensor.matmul(nb_p[:, ncol(k):ncol(k) + 44],
                                 sh_sb[:, TS * k:TS * (k + 1)],
                                 phys[:, 44 * t:44 * t + 44],
                                 start=(k in (0, 11)), stop=(k in (10, 16)),
                                 skip_group_check=True)
            nb = wrk.tile([P, 820], f32, tag="nb_sb")
            nc.scalar.copy(nb[:, 0:776], nb_p[:, 0:776])
            pos_c = nb[:, ncol(8) + 32:ncol(8) + 35]       # k=8 center
            ori_c = nb[:, ncol(8) + 35:ncol(8) + 44]

            def kview(k0, kn, off, width):
                # [(k: step 44, kn), (c: step 1, width)] view at block k0+off
                return nb[:, ncol(k0) + off:ncol(k0) + off + 44 * kn] \
                    .rearrange("p (k c) -> p k c", c=44)[:, :, 0:width]

            # ---- geometry -> delta_aug [P, (k,8)] ------------------------
            da = wrk.tile([P, K17 * 8], f32, tag="da")
            dav = da[:].rearrange("p (k d) -> p k d", d=8)
            D = wrk.tile([P, K17 * 3], f32, tag="D")
            Dv = D[:].rearrange("p (k a) -> p k a", a=3)
            nc.vector.tensor_sub(
                Dv[:, 0:11, :], kview(0, 11, 32, 3),
                pos_c.unsqueeze(1).broadcast_to([P, 11, 3]))
            nc.vector.tensor_sub(
                Dv[:, 11:17, :], kview(11, 6, 32, 3),
                pos_c.unsqueeze(1).broadcast_to([P, 6, 3]))
            sq = wrk.tile([P, K17 * 3], f32, tag="sq")
            nc.vector.tensor_mul(sq[:], D[:], D[:])
            d2 = wrk.tile([P, K17], f32, tag="d2")
            nc.vector.tensor_reduce(d2[:], sq[:].rearrange("p (k a) -> p k a", a=3),
                                    axis=AX.X, op=OP.add)
            # dist/R into delta slot 6 ; raw dist for direction
            nc.scalar.activation(dav[:, :, 6], d2[:], AF.Sqrt, bias=0.0,
                                 scale=1.0 / (R * R))
            dist = wrk.tile([P, K17], f32, tag="dist")
            nc.scalar.activation(dist[:], d2[:], AF.Sqrt, bias=0.0, scale=1.0)
            rec = wrk.tile([P, K17], f32, tag="rec")
            nc.vector.tensor_scalar_add(dist[:], dist[:], 1e-9)
            nc.vector.reciprocal(rec[:], dist[:])
            dirn = wrk.tile([P, K17 * 3], f32, tag="dirn")
            dirnv = dirn[:].rearrange("p (k a) -> p k a", a=3)
            nc.vector.tensor_mul(dirnv, Dv,
                                 rec[:].unsqueeze(-1).broadcast_to([P, K17, 3]))
            # local_a = sum_b Ri[a,b] * dirn[b]
            lm = wrk.tile([P, K17 * 9], f32, tag="lm")
            lmv = lm[:].rearrange("p (k a b) -> p k a b", a=3, b=3)
            nc.vector.tensor_mul(
                lmv,
                ori_c.rearrange("p (a b) -> p a b", b=3).unsqueeze(1)
                     .broadcast_to([P, K17, 3, 3]),
                dirn[:].rearrange("p (k b) -> p k b", b=3).unsqueeze(2)
                       .broadcast_to([P, K17, 3, 3]))
            nc.vector.tensor_reduce(dav[:, :, 0:3], lmv, axis=AX.X, op=OP.add)
            # ofeat_a = sum_b Ri[a,b] * Rj[a,b]
            ofm = wrk.tile([P, K17 * 9], f32, tag="ofm")
            ofmv = ofm[:].rearrange("p (k a b) -> p k a b", a=3, b=3)
            nc.vector.tensor_mul(
                ofmv[:, 0:11],
                kview(0, 11, 35, 9).rearrange("p k (a b) -> p k a b", b=3),
                ori_c.rearrange("p (a b) -> p a b", b=3).unsqueeze(1)
                     .broadcast_to([P, 11, 3, 3]))
            nc.vector.tensor_mul(
                ofmv[:, 11:17],
                kview(11, 6, 35, 9).rearrange("p k (a b) -> p k a b", b=3),
                ori_c.rearrange("p (a b) -> p a b", b=3).unsqueeze(1)
                     .broadcast_to([P, 6, 3, 3]))
            nc.vector.tensor_reduce(dav[:, :, 3:6], ofmv, axis=AX.X, op=OP.add)
            nc.vector.memset(dav[:, :, 7], 1.0)
            # chain-boundary mask (zeroes whole delta rows incl. bias)
            nc.vector.tensor_mul(
                dav, dav,
                mask_sb[0:P, K17 * t:K17 * (t + 1)].unsqueeze(-1)
                      .broadcast_to([P, K17, 8]))

            # ---- kern = lrelu(delta_aug @ WS, 0.2) -----------------------
            dT_p = ps.tile([128, 224], f32, tag="dT")
            nc.tensor.matmul(dT_p[:, 0:P], da[:, 0:128], id_sb[0:P, 0:P],
                             is_transpose=True, start=True, stop=False,
                             skip_group_check=True)
            nc.tensor.matmul(dT_p[0:8, P:P + P], da[:, 128:136], id_sb[0:P, 0:P],
                             is_transpose=True, start=False, stop=True,
                             skip_group_check=True)
            dT = wrk.tile([128, 224], f32, tag="dT_sb")
            nc.scalar.copy(dT[:], dT_p[:])
            pre_p = ps.tile([P, K17 * KC], f32, tag="pre")
            nc.tensor.matmul(pre_p[:], dT[:, 0:P], ws_a_sb[:], start=True,
                             stop=False, skip_group_check=True)
            nc.tensor.matmul(pre_p[:], dT[0:8, P:P + P], ws_b_sb[:], start=False,
                             stop=True, skip_group_check=True)
            kern = wrk.tile([P, K17 * KC], f32, tag="kern")
            nc.scalar.activation(kern[:], pre_p[:], AF.Prelu, bias=0.0,
                                 scale=1.0, alpha=a2_sb[0:P, 0:1])

            # ---- self-edge compensation into kern k=8 block --------------
            rn = wrk.tile([P, 3], f32, tag="rn")
            nc.vector.tensor_reduce(
                rn[:], ofm[:, 72:81].rearrange("p (a b) -> p a b", b=3),
                axis=AX.X, op=OP.add)
            pself = wrk.tile([P, KC], f32, tag="pself")
            nc.vector.scalar_tensor_tensor(pself[:], w5r_sb[0:P, 0:KC],
                                           rn[:, 0:1], b5r_sb[0:P, :],
                                           OP.mult, OP.add)
            nc.vector.scalar_tensor_tensor(pself[:], w5r_sb[0:P, KC:2 * KC],
                                           rn[:, 1:2], pself[:], OP.mult, OP.add)
            nc.vector.scalar_tensor_tensor(pself[:], w5r_sb[0:P, 2 * KC:3 * KC],
                                           rn[:, 2:3], pself[:], OP.mult, OP.add)
            kself = wrk.tile([P, KC], f32, tag="kself")
            nc.vector.scalar_tensor_tensor(kself[:], pself[:], NEG_K, pself[:],
                                           OP.mult, OP.max)
            nc.vector.tensor_scalar_mul(kself[:], kself[:], ncl_sb[0:P, t:t + 1])
            nc.gpsimd.tensor_add(kern[:, 8 * KC:9 * KC],
                                 kern[:, 8 * KC:9 * KC], kself[:])

            # ---- bilinear: tmp_k = kern_k (x) h_shift_k; PE transp-accum -
            aggT_p = ps.tile([128, 768], f32, tag="aggT")

            def tmp_mult(k, tag, eng):
                tm = tpool.tile([P, KC * W], bf16, tag=tag)
                eng.tensor_mul(
                    tm[:].rearrange("p (c w) -> p c w", w=W),
                    nb[:, ncol(k):ncol(k) + W].unsqueeze(1)
                      .broadcast_to([P, KC, W]),
                    kern[:, KC * k:KC * (k + 1)].unsqueeze(-1)
                        .broadcast_to([P, KC, W]))
                return tm

            def tmp_transp(k, tm):
                for b in range(6):
                    nc.tensor.matmul(
                        aggT_p[:, 128 * b:128 * b + P],
                        tm[:, 128 * b:128 * (b + 1)], idb_sb[0:P, 0:P],
                        start=(k == 0 and b in (0, 4)),
                        stop=(k == 16 and b in (3, 5)),
                        skip_group_check=True)

            for k in range(K17):
                tmp_transp(k, tmp_mult(k, "tmp", nc.vector))
            aggT = wrk.tile([128, 768], f32, tag="aggT_sb")
            nc.scalar.copy(aggT[:], aggT_p[:])

            # ---- conv = lrelu(agg @ Wk, 0.1) ; out = conv @ W_out + x ----
            co_p = ps2.tile([P, 240], f32, tag="co")
            for b in range(6):
                nc.tensor.matmul(co_p[0:W, 0:P], wk_sb[:, W * b:W * (b + 1)],
                                 aggT[:, 128 * b:128 * b + P],
                                 start=(b == 0), stop=(b == 5),
                                 skip_group_check=True)
            convL = wrk.tile([W, P], f32, tag="convL")
            nc.scalar.activation(convL[:], co_p[0:W, 0:P], AF.Prelu, bias=0.0,
                                 scale=1.0, alpha=a1_sb[0:W, 0:1])
            # start=True: zeroes this bank on partitions 0..111 (convT results
            # already consumed by the Prelu above; zeroing is per-partition-range)
            nc.tensor.matmul(co_p[:, P:P + 128], convL[:], w_out_sb[:],
                             start=True, stop=True, skip_group_check=True)
            out_sb = wrk.tile([P, C], f32, tag="out_sb")
            nc.vector.tensor_add(out_sb[:], co_p[:, P:P + 128],
                                 xc_all[0:P, C * t:C * t + C])
            cnt = min(TS, NPC - TS * t)
            nc.sync.dma_start(y[TS * t:TS * t + cnt, :], out_sb[0:cnt, :])

    nc.compile()
    return nc


def _expected_src_dst():
    i = np.arange(N)
    offs = np.arange(-WIN, WIN + 1)
    j = i[:, None] + offs[None, :]
    valid = ((j // L) == (i[:, None] // L)) & (j >= 0) & (j < N)
    j = np.where(valid, j, i[:, None])
    dst = np.repeat(i, offs.size).astype(np.int32)
    src = j.reshape(-1).astype(np.int32)
    return src, dst


def _host_inputs(x, pos, ori, W_in, Ws0, bs0, Wk, W_out):
    xf = np.ascontiguousarray(x.reshape(N, C), np.float32)
    pos = np.asarray(pos, np.float32)
    ori = np.asarray(ori, np.float32)

    # shared weights / constants
    WS = np.zeros((136, K17 * KC), np.float32)
    for k in range(K17):
        s = _sidx(k)
        WS[8 * k:8 * k + 7, KC * k:KC * (k + 1)] = Ws0[s]
        WS[8 * k + 7, KC * k:KC * (k + 1)] = bs0[s]
    wk_p = np.zeros((128, 6 * W), np.float32)
    for b in range(6):
        wk_p[:, W * b:W * (b + 1)] = Wk[128 * b:128 * (b + 1), :]
    shifts = np.zeros((128, K17 * TS), np.float32)
    for k in range(K17):
        for p in range(TS):
            shifts[p + k, TS * k + p] = 1.0
    w5r = np.tile(Ws0[5][3:6].reshape(1, 3 * KC), (128, 1)).astype(np.float32)
    b5r = np.tile(bs0[5].reshape(1, KC), (128, 1)).astype(np.float32)
    common = dict(
        w_in=np.ascontiguousarray(W_in, np.float32),
        ws_a=np.ascontiguousarray(WS[0:128]),
        ws_b=np.ascontiguousarray(WS[128:136]),
        wk_p=wk_p,
        w_out=np.ascontiguousarray(W_out, np.float32),
        ident=np.eye(128, dtype=np.float32),
        shifts=shifts,
        w5r=w5r, b5r=b5r,
        alph1=np.full((128, 1), NEG_IN, np.float32),
        alph2=np.full((128, 1), NEG_K, np.float32),
    )

    in_maps = []
    for ci in range(NCORES):
        s0 = ci * NPC
        g = s0 - WIN + np.arange(HR)
        ok = (g >= 0) & (g < N)
        gi = np.clip(g, 0, N - 1)
        x_pad = np.where(ok[:, None], xf[gi], 0.0).astype(np.float32)
        p_pad = np.where(ok[:, None], pos[gi], 0.0).astype(np.float32)
        o_pad = np.where(ok[:, None], ori[gi], 0.0).astype(np.float32)

        jj, pp = np.meshgrid(np.arange(NT), np.arange(128), indexing="ij")
        rows = (TS * jj + pp)            # [NT, 128] all < HR
        x_slot = x_pad[rows].transpose(1, 0, 2).reshape(128, NT * C)
        po_pad = np.concatenate([p_pad, o_pad], axis=1)  # [HR, 12]
        po_slot = po_pad[rows].transpose(1, 0, 2).reshape(128, NT * 12)
        rc = WIN + TS * jj + pp
        okc = rc < HR
        xc_slot = np.where(okc[:, :, None], x_pad[np.minimum(rc, HR - 1)], 0.0)
        xc_slot = xc_slot.transpose(1, 0, 2).reshape(128, NT * C).astype(np.float32)

        mask = np.zeros((128, NT, K17), np.float32)
        ncl = np.zeros((128, NT), np.float32)
        for t in range(NT):
            for p in range(min(TS, NPC - TS * t) if TS * t < NPC else 0):
                n = s0 + TS * t + p
                off = n % L
                v = ((off + np.arange(-WIN, WIN + 1)) >= 0) & \
                    ((off + np.arange(-WIN, WIN + 1)) < L)
                mask[p, t, :] = v.astype(np.float32)
                ncl[p, t] = K17 - v.sum()
        in_maps.append(dict(
            x_slot=x_slot, xc_slot=xc_slot, po_slot=po_slot,
            maskd=mask.reshape(128, NT * K17), nclmp=ncl, **common))
    return in_maps


def kernel(x, pos, seq, ori, W_in, Ws0, bs0, Wk, W_out, src, dst):
    exp_src, exp_dst = _expected_src_dst()
    assert np.array_equal(np.asarray(src), exp_src), "unexpected src graph"
    assert np.array_equal(np.asarray(dst), exp_dst), "unexpected dst graph"

    from concourse.bass_utils import run_bass_kernel_spmd

    if "nc" not in _PROG:
        _PROG["nc"] = _build_program()
    nc = _PROG["nc"]

    in_maps = _host_inputs(np.asarray(x), np.asarray(pos), np.asarray(ori),
                           np.asarray(W_in), np.asarray(Ws0), np.asarray(bs0),
                           np.asarray(Wk), np.asarray(W_out))
    res = run_bass_kernel_spmd(nc, in_maps, list(range(NCORES)))
    out = np.concatenate([res.results[i]["y"] for i in range(NCORES)], axis=0)
    return out.reshape(B, L, C).astype(np.float32)



# revision 4
# speedup vs baseline: 1.2690x; 1.2690x over previous
"""Bass/Trainium2 kernel for nn_BasicBlock_73933567033945 (CDConv / gnn_message_passing).

Strategy: graph is a fixed +-8 sequence window inside each of 4 chains of
L=2048 nodes (verified against the src/dst inputs at runtime). Shard the
8192 nodes across 8 NeuronCores (1024 nodes each) with an 8-node halo.

Per core:
  Phase A: hT[w, m] = lrelu(W_in^T @ lrelu(x)^T) computed directly in
    transposed form from a host-transposed x slab (no PE transposes).
  Phase A2: G[m, o*24+c] = sum_w hT[w, m] * Wk[c*32+w, o] (PE matmuls,
    hT slices stationary). G folds the output projection Wk into the
    gathered features, so the per-edge bilinear becomes
      conv[n, o] = sum_{k,c} kern[n, k, c] * G[n+k, o*24+c]
    (G slab row m of tile t holds node 112t + m - 8).
  Phase B per 112-node tile, products anchored at the G partition m:
    geometry is computed *center-shifted* -- the center pos|ori are
    gathered with down-shift matmuls so partition m holds delta of edge
    (n=m-k, k); the block-diagonal WS matmul then yields
    kern2[m, (k,c)] = kern(m-k, k, c) with no extra shifting. The DVE
    runs 17 all-bf16 contiguous multiplies tm_k[m] = G[m] * kern2[m,k],
    and 17 shift matmuls accumulate tm_k into PSUM re-aligned to the
    output node (out[n] += tm_k[n+k]), fusing the k-sum into the PE.
    A single 24-wide grouped reduce finishes the c-contraction.
    No per-k transposes, no Wk matmuls, no DVE add chains.
"""
import numpy as np

B, L, C = 4, 2048, 128
N = B * L
W = 32
KC = 24
SEQ_L = 11
R = 12.0
WIN = 8
NEG_IN = 0.1
NEG_K = 0.2
NCORES = 8
NPC = N // NCORES          # 1024 nodes per core
TS = 112                   # output nodes per tile
NT = 10                    # tiles per core (9*112 + 16)
HALO = 1152                # padded halo rows per core (needs 1136)
K17 = 2 * WIN + 1          # 17 window offsets
S_HALF = SEQ_L // 2
GW = W * KC                # 768 = width of a G slab / tm row

_PROG = {}


def _sidx(k):
    return int(np.clip(k - WIN, -S_HALF, S_HALF)) + S_HALF


def _build_program():
    import concourse.tile as tile
    from concourse import mybir, bacc
    from concourse.bass_utils import run_bass_kernel_spmd  # noqa: F401
    from contextlib import ExitStack

    f32 = mybir.dt.float32
    bf16 = mybir.dt.bfloat16
    AF = mybir.ActivationFunctionType
    OP = mybir.AluOpType
    AX = mybir.AxisListType

    nc = bacc.Bacc("TRN2", target_bir_lowering=False, debug=False)

    def din(name, shape):
        return nc.dram_tensor(name, shape, f32, kind="ExternalInput").ap()

    xT_slot = din("xT_slot", [128, HALO])
    xc_slot = din("xc_slot", [128, NT * C])
    po_slot = din("po_slot", [128, NT * 12])
    w_in = din("w_in", [C, W])
    ws_a = din("ws_a", [128, K17 * KC])
    ws_b = din("ws_b", [8, K17 * KC])
    wkoc = din("wkoc", [W, GW])
    w_out = din("w_out", [W, C])
    ident = din("ident", [128, 128])
    shifts_c = din("shifts_c", [128, K17 * 128])
    shifts_s = din("shifts_s", [128, K17 * TS])
    w5r = din("w5r", [128, 3 * KC])
    b5r = din("b5r", [128, KC])
    maskd = din("maskd", [128, NT * K17])
    nclmp = din("nclmp", [128, NT])
    alph1 = din("alph1", [128, 1])
    alph2 = din("alph2", [128, 1])
    y = nc.dram_tensor("y", [NPC, C], f32, kind="ExternalOutput").ap()

    with tile.TileContext(nc) as tc, ExitStack() as ctx:
        pers = ctx.enter_context(tc.tile_pool(name="pers", bufs=1))

        def load(ap_in, shape, tag):
            t = pers.tile(shape, f32, tag=tag)
            nc.sync.dma_start(t[:], ap_in)
            return t

        xT_sb = load(xT_slot, [128, HALO], "xT")
        xc_all = load(xc_slot, [128, NT * C], "xc_all")
        po_all = load(po_slot, [128, NT * 12], "po_all")
        w_in_f = load(w_in, [C, W], "w_in")
        ws_a_f = load(ws_a, [128, K17 * KC], "ws_a")
        ws_b_f = load(ws_b, [8, K17 * KC], "ws_b")
        wkoc_f = load(wkoc, [W, GW], "wkoc")
        w_out_f = load(w_out, [W, C], "w_out")
        id_sb = load(ident, [128, 128], "ident")
        shc_sb = load(shifts_c, [128, K17 * 128], "shifts_c")
        shs_f = load(shifts_s, [128, K17 * TS], "shifts_s")
        w5r_sb = load(w5r, [128, 3 * KC], "w5r")
        b5r_sb = load(b5r, [128, KC], "b5r")
        mask_sb = load(maskd, [128, NT * K17], "mask")
        ncl_sb = load(nclmp, [128, NT], "nclmp")
        a1_sb = load(alph1, [128, 1], "a1")
        a2_sb = load(alph2, [128, 1], "a2")

        # bf16 casts of shared weights
        w_in_b = pers.tile([C, W], bf16, tag="w_in_b")
        nc.vector.tensor_copy(w_in_b[:], w_in_f[:])
        ws_a_b = pers.tile([128, K17 * KC], bf16, tag="ws_a_b")
        nc.vector.tensor_copy(ws_a_b[:], ws_a_f[:])
        ws_b_b = pers.tile([8, K17 * KC], bf16, tag="ws_b_b")
        nc.vector.tensor_copy(ws_b_b[:], ws_b_f[:])
        wkoc_b = pers.tile([W, GW], bf16, tag="wkoc_b")
        nc.vector.tensor_copy(wkoc_b[:], wkoc_f[:])
        w_out_b = pers.tile([W, C], bf16, tag="w_out_b")
        nc.vector.tensor_copy(w_out_b[:], w_out_f[:])
        shs_b = pers.tile([128, K17 * TS], bf16, tag="shs_b")
        nc.vector.tensor_copy(shs_b[:], shs_f[:])

        hT = pers.tile([W, HALO], bf16, tag="hT")
        G = pers.tile([128, NT * GW], bf16, tag="G")

        # ---------------- Phase A: hT = lrelu(W_in^T @ lrelu(x)^T) ---------
        with tc.tile_pool(name="pA", bufs=2) as pA, \
             tc.tile_pool(name="pAp", bufs=2, space="PSUM") as pAp:
            xlT = pA.tile([128, HALO], bf16, tag="xlT")
            for s in range(3):
                sl = slice(384 * s, 384 * (s + 1))
                nc.scalar.activation(xlT[:, sl], xT_sb[:, sl], AF.Prelu,
                                     bias=0.0, scale=1.0, alpha=a1_sb[:, 0:1])
                hp = pAp.tile([W, 384], f32, tag="hp")
                nc.tensor.matmul(hp[:], w_in_b[:], xlT[:, sl],
                                 start=True, stop=True)
                nc.scalar.activation(hT[:, sl], hp[:], AF.Prelu,
                                     bias=0.0, scale=1.0, alpha=a1_sb[0:W, 0:1])

        # ---------------- Phase A2: G slabs = hT_slice^T @ Wkoc ------------
        with tc.tile_pool(name="pG", bufs=2, space="PSUM") as pG:
            for t in range(NT):
                gp = pG.tile([128, GW], f32, tag="gp")
                nc.tensor.matmul(gp[:, 0:512], hT[:, TS * t:TS * t + 128],
                                 wkoc_b[:, 0:512], start=True, stop=True,
                                 skip_group_check=True)
                nc.tensor.matmul(gp[:, 512:GW], hT[:, TS * t:TS * t + 128],
                                 wkoc_b[:, 512:GW], start=True, stop=True,
                                 skip_group_check=True)
                if t % 2 == 0:
                    nc.scalar.copy(G[:, GW * t:GW * (t + 1)], gp[:])
                else:
                    nc.vector.tensor_copy(G[:, GW * t:GW * (t + 1)], gp[:])

        # ---------------- Phase B: per output tile ------------------------
        wrk = ctx.enter_context(tc.tile_pool(name="wrk", bufs=2))
        tpool = ctx.enter_context(tc.tile_pool(name="tmp", bufs=3))
        ps = ctx.enter_context(tc.tile_pool(name="ps", bufs=1, space="PSUM"))
        ps2 = ctx.enter_context(tc.tile_pool(name="ps2", bufs=1, space="PSUM"))

        P = 128  # products/geometry live on all 128 partitions (m = n + k)
        for t in range(NT):
            # ---- center pos|ori gather: nbc[m, k] = po[m + 8 - k] --------
            nb_ps = ps.tile([P, K17 * 12], f32, tag="nb")
            for k in range(K17):
                nc.tensor.matmul(nb_ps[:, 12 * k:12 * (k + 1)],
                                 shc_sb[:, 128 * k:128 * (k + 1)],
                                 po_all[:, 12 * t:12 * (t + 1)],
                                 start=(k == 0), stop=(k == K17 - 1),
                                 skip_group_check=True)
            nb = wrk.tile([P, K17 * 12], f32, tag="nb_sb")
            nc.scalar.copy(nb[:], nb_ps[:])
            nbv = nb[:].rearrange("p (k d) -> p k d", d=12)
            poN = po_all[:, 12 * t:12 * (t + 1)]     # neighbor = po[m]
            poN_pos = poN[:, 0:3]
            poN_ori = poN[:, 3:12]

            # ---- geometry -> delta_aug [P, (k,8)] for edge (m-k, k) ------
            da = wrk.tile([P, K17 * 8], f32, tag="da")
            dav = da[:].rearrange("p (k d) -> p k d", d=8)
            D = wrk.tile([P, K17 * 3], f32, tag="D")
            Dv = D[:].rearrange("p (k a) -> p k a", a=3)
            nc.vector.tensor_sub(Dv,
                                 poN_pos.unsqueeze(1).broadcast_to([P, K17, 3]),
                                 nbv[:, :, 0:3])
            sq = wrk.tile([P, K17 * 3], f32, tag="sq")
            nc.vector.tensor_mul(sq[:], D[:], D[:])
            d2 = wrk.tile([P, K17], f32, tag="d2")
            nc.vector.tensor_reduce(d2[:], sq[:].rearrange("p (k a) -> p k a", a=3),
                                    axis=AX.X, op=OP.add)
            nc.scalar.activation(dav[:, :, 6], d2[:], AF.Sqrt, bias=0.0,
                                 scale=1.0 / (R * R))
            dist = wrk.tile([P, K17], f32, tag="dist")
            nc.scalar.activation(dist[:], d2[:], AF.Sqrt, bias=0.0, scale=1.0)
            rec = wrk.tile([P, K17], f32, tag="rec")
            nc.vector.tensor_scalar_add(dist[:], dist[:], 1e-9)
            nc.vector.reciprocal(rec[:], dist[:])
            dirn = wrk.tile([P, K17 * 3], f32, tag="dirn")
            dirnv = dirn[:].rearrange("p (k a) -> p k a", a=3)
            nc.vector.tensor_mul(dirnv, Dv,
                                 rec[:].unsqueeze(-1).broadcast_to([P, K17, 3]))
            # local_a = sum_b Ri[a,b] * dirn[b]; Ri = center ori (from nbc)
            lm = wrk.tile([P, K17 * 9], f32, tag="lm")
            lmv = lm[:].rearrange("p (k a b) -> p k a b", a=3, b=3)
            nc.vector.tensor_mul(
                lmv,
                nbv[:, :, 3:12].rearrange("p k (a b) -> p k a b", b=3),
                dirn[:].rearrange("p (k b) -> p k b", b=3).unsqueeze(2)
                       .broadcast_to([P, K17, 3, 3]))
            nc.vector.tensor_reduce(dav[:, :, 0:3], lmv, axis=AX.X, op=OP.add)
            # ofeat_a = sum_b Ri[a,b] * Rj[a,b]; Rj = neighbor ori (= po[m])
            ofm = wrk.tile([P, K17 * 9], f32, tag="ofm")
            ofmv = ofm[:].rearrange("p (k a b) -> p k a b", a=3, b=3)
            nc.vector.tensor_mul(
                ofmv,
                nbv[:, :, 3:12].rearrange("p k (a b) -> p k a b", b=3),
                poN_ori.rearrange("p (a b) -> p a b", b=3).unsqueeze(1)
                       .broadcast_to([P, K17, 3, 3]))
            nc.vector.tensor_reduce(dav[:, :, 3:6], ofmv, axis=AX.X, op=OP.add)
            nc.vector.memset(dav[:, :, 7], 1.0)
            nc.vector.tensor_mul(
                dav, dav,
                mask_sb[:, K17 * t:K17 * (t + 1)].unsqueeze(-1)
                      .broadcast_to([P, K17, 8]))

            # ---- kern2[m,(k,c)] = lrelu(delta @ WS) = kern(m-k, k, c) ----
            dT_ps = ps.tile([128, 256], f32, tag="dT")
            nc.tensor.matmul(dT_ps[:, 0:128], da[:, 0:128], id_sb[:, :],
                             is_transpose=True, start=True, stop=False,
                             skip_group_check=True)
            nc.tensor.matmul(dT_ps[0:8, 128:256], da[:, 128:136], id_sb[:, :],
                             is_transpose=True, start=False, stop=True,
                             skip_group_check=True)
            dT = wrk.tile([128, 256], bf16, tag="dT_sb")
            nc.scalar.copy(dT[:], dT_ps[:])
            pre_ps = ps.tile([P, K17 * KC], f32, tag="pre")
            nc.tensor.matmul(pre_ps[:], dT[:, 0:128], ws_a_b[:], start=True,
                             stop=False, skip_group_check=True)
            nc.tensor.matmul(pre_ps[:], dT[0:8, 128:256], ws_b_b[:], start=False,
                             stop=True, skip_group_check=True)
            kern = wrk.tile([P, K17 * KC], bf16, tag="kern")
            nc.scalar.activation(kern[:], pre_ps[:], AF.Prelu, bias=0.0,
                                 scale=1.0, alpha=a2_sb[:, 0:1])

            # ---- self-edge compensation into kern k=8 block --------------
            rn = wrk.tile([P, 3], f32, tag="rn")
            nc.vector.tensor_reduce(
                rn[:], ofm[:, 72:81].rearrange("p (a b) -> p a b", b=3),
                axis=AX.X, op=OP.add)
            pself = wrk.tile([P, KC], f32, tag="pself")
            nc.vector.scalar_tensor_tensor(pself[:], w5r_sb[:, 0:KC],
                                           rn[:, 0:1], b5r_sb[:, :],
                                           OP.mult, OP.add)
            nc.vector.scalar_tensor_tensor(pself[:], w5r_sb[:, KC:2 * KC],
                                           rn[:, 1:2], pself[:], OP.mult, OP.add)
            nc.vector.scalar_tensor_tensor(pself[:], w5r_sb[:, 2 * KC:3 * KC],
                                           rn[:, 2:3], pself[:], OP.mult, OP.add)
            kself = wrk.tile([P, KC], f32, tag="kself")
            nc.vector.scalar_tensor_tensor(kself[:], pself[:], NEG_K, pself[:],
                                           OP.mult, OP.max)
            kself_b = wrk.tile([P, KC], bf16, tag="kself_b")
            nc.vector.tensor_scalar_mul(kself_b[:], kself[:], ncl_sb[:, t:t + 1])
            nc.gpsimd.tensor_add(kern[:, 8 * KC:9 * KC],
                                 kern[:, 8 * KC:9 * KC], kself_b[:])

            # ---- tm_k[m] = G[m] * kern2[m, k]; PE shift-accumulate -------
            wide_ps = ps.tile([TS, GW], f32, tag="wide")
            for k in range(K17):
                tm = tpool.tile([P, GW], bf16, tag="tm")
                nc.vector.tensor_mul(
                    tm[:].rearrange("p (o c) -> p o c", c=KC),
                    G[:, GW * t:GW * (t + 1)]
                        .rearrange("p (o c) -> p o c", c=KC),
                    kern[:, KC * k:KC * (k + 1)].unsqueeze(1)
                        .broadcast_to([P, W, KC]))
                nc.tensor.matmul(wide_ps[:, 0:512],
                                 shs_b[:, TS * k:TS * (k + 1)], tm[:, 0:512],
                                 start=(k == 0), stop=(k == K17 - 1),
                                 skip_group_check=True)
                nc.tensor.matmul(wide_ps[:, 512:GW],
                                 shs_b[:, TS * k:TS * (k + 1)], tm[:, 512:GW],
                                 start=(k == 0), stop=(k == K17 - 1),
                                 skip_group_check=True)
            conv = wrk.tile([TS, W], f32, tag="conv")
            nc.vector.tensor_reduce(conv[:],
                                    wide_ps[:].rearrange("p (o c) -> p o c", c=KC),
                                    axis=AX.X, op=OP.add)

            # ---- out = lrelu(conv) @ W_out + x ---------------------------
            ct_ps = ps2.tile([W, TS], f32, tag="ct")
            nc.tensor.matmul(ct_ps[:], conv[:], id_sb[0:TS, 0:TS],
                             is_transpose=True, start=True, stop=True,
                             skip_group_check=True)
            convLT = wrk.tile([W, TS], bf16, tag="convLT")
            nc.scalar.activation(convLT[:], ct_ps[:], AF.Prelu, bias=0.0,
                                 scale=1.0, alpha=a1_sb[0:W, 0:1])
            out_ps = ps2.tile([TS, C], f32, tag="out")
            nc.tensor.matmul(out_ps[:], convLT[:], w_out_b[:],
                             start=True, stop=True, skip_group_check=True)
            out_sb = wrk.tile([TS, C], f32, tag="out_sb")
            nc.vector.tensor_add(out_sb[:], out_ps[:],
                                 xc_all[0:TS, C * t:C * t + C])
            cnt = min(TS, NPC - TS * t)
            nc.sync.dma_start(y[TS * t:TS * t + cnt, :], out_sb[0:cnt, :])

    nc.compile()
    return nc


def _expected_src_dst():
    i = np.arange(N)
    offs = np.arange(-WIN, WIN + 1)
    j = i[:, None] + offs[None, :]
    valid = ((j // L) == (i[:, None] // L)) & (j >= 0) & (j < N)
    j = np.where(valid, j, i[:, None])
    dst = np.repeat(i, offs.size).astype(np.int32)
    src = j.reshape(-1).astype(np.int32)
    return src, dst


def _host_inputs(x, pos, ori, W_in, Ws0, bs0, Wk, W_out):
    xf = np.ascontiguousarray(x.reshape(N, C), np.float32)
    pos = np.asarray(pos, np.float32)
    ori = np.asarray(ori, np.float32)

    WS = np.zeros((136, K17 * KC), np.float32)
    for k in range(K17):
        s = _sidx(k)
        WS[8 * k:8 * k + 7, KC * k:KC * (k + 1)] = Ws0[s]
        WS[8 * k + 7, KC * k:KC * (k + 1)] = bs0[s]
    # wkoc[w, o*KC + c] = Wk[c*W + w, o]
    wkoc = np.ascontiguousarray(
        np.transpose(np.asarray(Wk, np.float32).reshape(KC, W, W),
                     (1, 2, 0)).reshape(W, GW))
    # center gather: nbc[m, k] = po[m + 8 - k]
    shifts_c = np.zeros((128, K17 * 128), np.float32)
    for k in range(K17):
        for m in range(128):
            r = m + 8 - k
            if 0 <= r < 128:
                shifts_c[r, 128 * k + m] = 1.0
    # shift-accumulate: out[n] += tm_k[n + k]
    shifts_s = np.zeros((128, K17 * TS), np.float32)
    for k in range(K17):
        for n in range(TS):
            shifts_s[n + k, TS * k + n] = 1.0
    w5r = np.tile(Ws0[5][3:6].reshape(1, 3 * KC), (128, 1)).astype(np.float32)
    b5r = np.tile(bs0[5].reshape(1, KC), (128, 1)).astype(np.float32)
    common = dict(
        w_in=np.ascontiguousarray(W_in, np.float32),
        ws_a=np.ascontiguousarray(WS[0:128]),
        ws_b=np.ascontiguousarray(WS[128:136]),
        wkoc=wkoc,
        w_out=np.ascontiguousarray(W_out, np.float32),
        ident=np.eye(128, dtype=np.float32),
        shifts_c=shifts_c,
        shifts_s=shifts_s,
        w5r=w5r, b5r=b5r,
        alph1=np.full((128, 1), NEG_IN, np.float32),
        alph2=np.full((128, 1), NEG_K, np.float32),
    )

    offs = np.arange(-WIN, WIN + 1)
    in_maps = []
    for ci in range(NCORES):
        s0 = ci * NPC
        g = s0 - WIN + np.arange(HALO)
        ok = (g >= 0) & (g < N)
        gi = np.clip(g, 0, N - 1)
        x_pad = np.where(ok[:, None], xf[gi], 0.0).astype(np.float32)
        p_pad = np.where(ok[:, None], pos[gi], 0.0).astype(np.float32)
        o_pad = np.where(ok[:, None], ori[gi], 0.0).astype(np.float32)

        xT_slot = np.ascontiguousarray(x_pad.T)                # [128, HALO]

        jj, pp = np.meshgrid(np.arange(NT), np.arange(128), indexing="ij")
        rows = (TS * jj + pp)            # [NT, 128] all < HALO
        po_pad = np.concatenate([p_pad, o_pad], axis=1)  # [HALO, 12]
        po_slot = po_pad[rows].transpose(1, 0, 2).reshape(128, NT * 12)
        rc = WIN + TS * jj + pp
        okc = rc < HALO
        xc_slot = np.where(okc[:, :, None], x_pad[np.minimum(rc, HALO - 1)], 0.0)
        xc_slot = xc_slot.transpose(1, 0, 2).reshape(128, NT * C).astype(np.float32)

        # mask2[m, t, k]: edge (n = m-k, k) exists; ncl2[m, t]: #folded
        # self-loops of node m-8 (kern2/kself live at partition m = n + k)
        mask = np.zeros((128, NT, K17), np.float32)
        ncl = np.zeros((128, NT), np.float32)
        for t in range(NT):
            cnt = min(TS, NPC - TS * t)
            for m in range(128):
                for k in range(K17):
                    n = m - k
                    if 0 <= n < cnt:
                        off = (s0 + TS * t + n) % L
                        if 0 <= off + k - WIN < L:
                            mask[m, t, k] = 1.0
                nn = m - WIN
                if 0 <= nn < cnt:
                    off = (s0 + TS * t + nn) % L
                    v = ((off + offs) >= 0) & ((off + offs) < L)
                    ncl[m, t] = K17 - v.sum()
        in_maps.append(dict(
            xT_slot=xT_slot, xc_slot=xc_slot, po_slot=po_slot,
            maskd=mask.reshape(128, NT * K17), nclmp=ncl, **common))
    return in_maps


def kernel(x, pos, seq, ori, W_in, Ws0, bs0, Wk, W_out, src, dst):
    exp_src, exp_dst = _expected_src_dst()
    assert np.array_equal(np.asarray(src), exp_src), "unexpected src graph"
    assert np.array_equal(np.asarray(dst), exp_dst), "unexpected dst graph"

    from concourse.bass_utils import run_bass_kernel_spmd

    if "nc" not in _PROG:
        _PROG["nc"] = _build_program()
    nc = _PROG["nc"]

    in_maps = _host_inputs(np.asarray(x), np.asarray(pos), np.asarray(ori),
                           np.asarray(W_in), np.asarray(Ws0), np.asarray(bs0),
                           np.asarray(Wk), np.asarray(W_out))
    res = run_bass_kernel_spmd(nc, in_maps, list(range(NCORES)))
    out = np.concatenate([res.results[i]["y"] for i in range(NCORES)], axis=0)
    return out.reshape(B, L, C).astype(np.float32)


# revision 9
# speedup vs baseline: 1.6628x; 1.3103x over previous
"""Bass/Trainium2 kernel for nn_BasicBlock_73933567033945 (CDConv / gnn_message_passing).

Strategy: graph is a fixed +-8 sequence window inside each of 4 chains of
L=2048 nodes (verified against the src/dst inputs at runtime). Shard the
8192 nodes across 8 NeuronCores (1024 nodes each) with an 8-node halo.

Per core:
  Phase A: hT[w, m] = lrelu(W_in^T @ lrelu(x)^T) computed directly in
    transposed form from a host-transposed x slab (no PE transposes).
  Phase A2: G[m, o*24+c] = sum_w hT[w, m] * Wk[c*32+w, o] (PE matmuls,
    hT slices stationary). G folds the output projection Wk into the
    gathered features, so the per-edge bilinear becomes
      conv[n, o] = sum_{k,c} kern[n, k, c] * G[n+k, o*24+c]
    (G slab row m of tile t holds node 112t + m - 8).
  Phase B per 112-node tile, products anchored at the G partition m:
    geometry is computed *center-shifted* -- the center pos|ori are
    gathered with down-shift matmuls so partition m holds delta of edge
    (n=m-k, k); the block-diagonal WS matmul then yields
    kern2[m, (k,c)] = kern(m-k, k, c) with no extra shifting. The DVE
    runs 17 all-bf16 contiguous multiplies tm_k[m] = G[m] * kern2[m,k],
    and 17 shift matmuls accumulate tm_k into PSUM re-aligned to the
    output node (out[n] += tm_k[n+k]), fusing the k-sum into the PE.
    A single 24-wide grouped reduce finishes the c-contraction.
    No per-k transposes, no Wk matmuls, no DVE add chains.
"""
import numpy as np

B, L, C = 4, 2048, 128
N = B * L
W = 32
KC = 24
SEQ_L = 11
R = 12.0
WIN = 8
NEG_IN = 0.1
NEG_K = 0.2
NCORES = 8
NPC = N // NCORES          # 1024 nodes per core
TS = 112                   # output nodes per tile
NT = 10                    # tiles per core (9*112 + 16)
HALO = 1152                # padded halo rows per core (needs 1136)
K17 = 2 * WIN + 1          # 17 window offsets
S_HALF = SEQ_L // 2
GW = W * KC                # 768 = width of a G slab / tm row

_PROG = {}


def _sidx(k):
    return int(np.clip(k - WIN, -S_HALF, S_HALF)) + S_HALF


def _build_program():
    import concourse.tile as tile
    from concourse import mybir, bacc
    from concourse.bass_utils import run_bass_kernel_spmd  # noqa: F401
    from contextlib import ExitStack

    f32 = mybir.dt.float32
    bf16 = mybir.dt.bfloat16
    AF = mybir.ActivationFunctionType
    OP = mybir.AluOpType
    AX = mybir.AxisListType

    nc = bacc.Bacc("TRN2", target_bir_lowering=False, debug=False)

    def din(name, shape):
        return nc.dram_tensor(name, shape, f32, kind="ExternalInput").ap()

    xT_slot = din("xT_slot", [128, HALO])
    xc_slot = din("xc_slot", [128, NT * C])
    po_slot = din("po_slot", [128, NT * 12])
    w_in = din("w_in", [C, W])
    ws_a = din("ws_a", [128, K17 * KC])
    ws_b = din("ws_b", [8, K17 * KC])
    wkoc = din("wkoc", [W, GW])
    w_out = din("w_out", [W, C])
    ident = din("ident", [128, 128])
    shifts_c = din("shifts_c", [128, K17 * 128])
    shifts_s = din("shifts_s", [128, K17 * TS])
    w5r = din("w5r", [128, 3 * KC])
    b5r = din("b5r", [128, KC])
    maskd = din("maskd", [128, NT * K17])
    nclmp = din("nclmp", [128, NT])
    alph1 = din("alph1", [128, 1])
    alph2 = din("alph2", [128, 1])
    y = nc.dram_tensor("y", [NPC, C], f32, kind="ExternalOutput").ap()

    with tile.TileContext(nc) as tc, ExitStack() as ctx:
        pers = ctx.enter_context(tc.tile_pool(name="pers", bufs=1))

        def load(ap_in, shape, tag):
            t = pers.tile(shape, f32, tag=tag)
            nc.sync.dma_start(t[:], ap_in)
            return t

        xT_sb = load(xT_slot, [128, HALO], "xT")
        xc_all = load(xc_slot, [128, NT * C], "xc_all")
        po_all = load(po_slot, [128, NT * 12], "po_all")
        w_in_f = load(w_in, [C, W], "w_in")
        ws_a_f = load(ws_a, [128, K17 * KC], "ws_a")
        ws_b_f = load(ws_b, [8, K17 * KC], "ws_b")
        wkoc_f = load(wkoc, [W, GW], "wkoc")
        w_out_f = load(w_out, [W, C], "w_out")
        id_sb = load(ident, [128, 128], "ident")
        shc_sb = load(shifts_c, [128, K17 * 128], "shifts_c")
        shs_f = load(shifts_s, [128, K17 * TS], "shifts_s")
        w5r_sb = load(w5r, [128, 3 * KC], "w5r")
        b5r_sb = load(b5r, [128, KC], "b5r")
        mask_sb = load(maskd, [128, NT * K17], "mask")
        ncl_sb = load(nclmp, [128, NT], "nclmp")
        a1_sb = load(alph1, [128, 1], "a1")
        a2_sb = load(alph2, [128, 1], "a2")

        # bf16 casts of shared weights
        w_in_b = pers.tile([C, W], bf16, tag="w_in_b")
        nc.vector.tensor_copy(w_in_b[:], w_in_f[:])
        ws_a_b = pers.tile([128, K17 * KC], bf16, tag="ws_a_b")
        nc.vector.tensor_copy(ws_a_b[:], ws_a_f[:])
        ws_b_b = pers.tile([8, K17 * KC], bf16, tag="ws_b_b")
        nc.vector.tensor_copy(ws_b_b[:], ws_b_f[:])
        wkoc_b = pers.tile([W, GW], bf16, tag="wkoc_b")
        nc.vector.tensor_copy(wkoc_b[:], wkoc_f[:])
        w_out_b = pers.tile([W, C], bf16, tag="w_out_b")
        nc.vector.tensor_copy(w_out_b[:], w_out_f[:])
        shs_b = pers.tile([128, K17 * TS], bf16, tag="shs_b")
        nc.vector.tensor_copy(shs_b[:], shs_f[:])

        hT = pers.tile([W, HALO], bf16, tag="hT")
        G = pers.tile([128, NT * GW], bf16, tag="G")

        # ---------------- Phase A: hT = lrelu(W_in^T @ lrelu(x)^T) ---------
        with tc.tile_pool(name="pA", bufs=2) as pA, \
             tc.tile_pool(name="pAp", bufs=2, space="PSUM") as pAp:
            xlT = pA.tile([128, HALO], bf16, tag="xlT")
            for s in range(3):
                sl = slice(384 * s, 384 * (s + 1))
                nc.scalar.activation(xlT[:, sl], xT_sb[:, sl], AF.Prelu,
                                     bias=0.0, scale=1.0, alpha=a1_sb[:, 0:1])
                hp = pAp.tile([W, 384], f32, tag="hp")
                nc.tensor.matmul(hp[:], w_in_b[:], xlT[:, sl],
                                 start=True, stop=True)
                nc.scalar.activation(hT[:, sl], hp[:], AF.Prelu,
                                     bias=0.0, scale=1.0, alpha=a1_sb[0:W, 0:1])

        # ---------------- Phase A2: G slabs = hT_slice^T @ Wkoc ------------
        with tc.tile_pool(name="pG", bufs=2, space="PSUM") as pG:
            for t in range(NT):
                gp = pG.tile([128, GW], f32, tag="gp")
                nc.tensor.matmul(gp[:, 0:512], hT[:, TS * t:TS * t + 128],
                                 wkoc_b[:, 0:512], start=True, stop=True,
                                 skip_group_check=True)
                nc.tensor.matmul(gp[:, 512:GW], hT[:, TS * t:TS * t + 128],
                                 wkoc_b[:, 512:GW], start=True, stop=True,
                                 skip_group_check=True)
                if t % 2 == 0:
                    nc.scalar.copy(G[:, GW * t:GW * (t + 1)], gp[:])
                else:
                    nc.vector.tensor_copy(G[:, GW * t:GW * (t + 1)], gp[:])

        # ------ Phase A3: all tiles' center gathers nbc[m, k, t] ----------
        # nbc_all[m, 120*k + 12*t + d] = po[m + 8 - k, 12*t + d]
        nbc_all = pers.tile([128, K17 * NT * 12], f32, tag="nbc_all")
        with tc.tile_pool(name="pN", bufs=2, space="PSUM") as pN:
            for g in range(5):               # 4 k's per PSUM bank pass
                ks = range(4 * g, min(4 * g + 4, K17))
                np_ps = pN.tile([128, 480], f32, tag="np")
                for i, k in enumerate(ks):
                    nc.tensor.matmul(np_ps[:, 120 * i:120 * (i + 1)],
                                     shc_sb[:, 128 * k:128 * (k + 1)],
                                     po_all[:], start=True, stop=True,
                                     skip_group_check=True)
                nc.scalar.copy(nbc_all[:, 480 * g:480 * g + 120 * len(ks)],
                               np_ps[:, 0:120 * len(ks)])

        # ---------------- Phase B: per output tile ------------------------
        wrk = ctx.enter_context(tc.tile_pool(name="wrk", bufs=2))
        tpool = ctx.enter_context(tc.tile_pool(name="tmp", bufs=3))
        ps = ctx.enter_context(tc.tile_pool(name="ps", bufs=1, space="PSUM"))
        psw = ctx.enter_context(tc.tile_pool(name="psw", bufs=2, space="PSUM"))
        ps2 = ctx.enter_context(tc.tile_pool(name="ps2", bufs=1, space="PSUM"))

        P = 128  # products/geometry live on all 128 partitions (m = n + k)
        for t in range(NT):
            nbv = nbc_all[:].rearrange("p (k td) -> p k td", td=NT * 12) \
                            [:, :, 12 * t:12 * (t + 1)]
            poN = po_all[:, 12 * t:12 * (t + 1)]     # neighbor = po[m]
            poN_pos = poN[:, 0:3]
            poN_ori = poN[:, 3:12]

            # ---- geometry -> delta_aug [P, (k,8)] for edge (m-k, k) ------
            da = wrk.tile([P, K17 * 8], f32, tag="da")
            dav = da[:].rearrange("p (k d) -> p k d", d=8)
            D = wrk.tile([P, K17 * 3], f32, tag="D")
            Dv = D[:].rearrange("p (k a) -> p k a", a=3)
            nc.vector.tensor_sub(Dv,
                                 poN_pos.unsqueeze(1).broadcast_to([P, K17, 3]),
                                 nbv[:, :, 0:3])
            sq = wrk.tile([P, K17 * 3], f32, tag="sq")
            nc.vector.tensor_mul(sq[:], D[:], D[:])
            d2 = wrk.tile([P, K17], f32, tag="d2")
            nc.vector.tensor_reduce(d2[:], sq[:].rearrange("p (k a) -> p k a", a=3),
                                    axis=AX.X, op=OP.add)
            nc.scalar.activation(dav[:, :, 6], d2[:], AF.Sqrt, bias=0.0,
                                 scale=1.0 / (R * R))
            dist = wrk.tile([P, K17], f32, tag="dist")
            nc.scalar.activation(dist[:], d2[:], AF.Sqrt, bias=0.0, scale=1.0)
            rec = wrk.tile([P, K17], f32, tag="rec")
            nc.vector.tensor_scalar_add(dist[:], dist[:], 1e-9)
            nc.vector.reciprocal(rec[:], dist[:])
            dirn = wrk.tile([P, K17 * 3], f32, tag="dirn")
            dirnv = dirn[:].rearrange("p (k a) -> p k a", a=3)
            nc.vector.tensor_mul(dirnv, Dv,
                                 rec[:].unsqueeze(-1).broadcast_to([P, K17, 3]))
            # local_a = sum_b Ri[a,b] * dirn[b]; Ri = center ori (from nbc)
            lm = wrk.tile([P, K17 * 9], f32, tag="lm")
            lmv = lm[:].rearrange("p (k a b) -> p k a b", a=3, b=3)
            nc.gpsimd.tensor_mul(
                lmv,
                nbv[:, :, 3:12].rearrange("p k (a b) -> p k a b", b=3),
                dirn[:].rearrange("p (k b) -> p k b", b=3).unsqueeze(2)
                       .broadcast_to([P, K17, 3, 3]))
            nc.vector.tensor_reduce(dav[:, :, 0:3], lmv, axis=AX.X, op=OP.add)
            # ofeat_a = sum_b Ri[a,b] * Rj[a,b]; Rj = neighbor ori (= po[m])
            ofm = wrk.tile([P, K17 * 9], f32, tag="ofm")
            ofmv = ofm[:].rearrange("p (k a b) -> p k a b", a=3, b=3)
            nc.gpsimd.tensor_mul(
                ofmv,
                nbv[:, :, 3:12].rearrange("p k (a b) -> p k a b", b=3),
                poN_ori.rearrange("p (a b) -> p a b", b=3).unsqueeze(1)
                       .broadcast_to([P, K17, 3, 3]))
            nc.vector.tensor_reduce(dav[:, :, 3:6], ofmv, axis=AX.X, op=OP.add)
            nc.vector.memset(dav[:, :, 7], 1.0)
            nc.gpsimd.tensor_mul(
                dav, dav,
                mask_sb[:, K17 * t:K17 * (t + 1)].unsqueeze(-1)
                      .broadcast_to([P, K17, 8]))

            # ---- kern2[m,(k,c)] = lrelu(delta @ WS) = kern(m-k, k, c) ----
            dT_ps = ps.tile([128, 256], f32, tag="dT")
            nc.tensor.matmul(dT_ps[:, 0:128], da[:, 0:128], id_sb[:, :],
                             is_transpose=True, start=True, stop=False,
                             skip_group_check=True)
            nc.tensor.matmul(dT_ps[0:8, 128:256], da[:, 128:136], id_sb[:, :],
                             is_transpose=True, start=False, stop=True,
                             skip_group_check=True)
            dT = wrk.tile([128, 256], bf16, tag="dT_sb")
            nc.scalar.copy(dT[:], dT_ps[:])
            pre_ps = ps.tile([P, K17 * KC], f32, tag="pre")
            nc.tensor.matmul(pre_ps[:], dT[:, 0:128], ws_a_b[:], start=True,
                             stop=False, skip_group_check=True)
            nc.tensor.matmul(pre_ps[:], dT[0:8, 128:256], ws_b_b[:], start=False,
                             stop=True, skip_group_check=True)
            kern = wrk.tile([P, K17 * KC], bf16, tag="kern")
            nc.scalar.activation(kern[:], pre_ps[:], AF.Prelu, bias=0.0,
                                 scale=1.0, alpha=a2_sb[:, 0:1])

            # ---- self-edge compensation into kern k=8 block --------------
            rn = wrk.tile([P, 3], f32, tag="rn")
            nc.vector.tensor_reduce(
                rn[:], ofm[:, 72:81].rearrange("p (a b) -> p a b", b=3),
                axis=AX.X, op=OP.add)
            pself = wrk.tile([P, KC], f32, tag="pself")
            nc.vector.scalar_tensor_tensor(pself[:], w5r_sb[:, 0:KC],
                                           rn[:, 0:1], b5r_sb[:, :],
                                           OP.mult, OP.add)
            nc.vector.scalar_tensor_tensor(pself[:], w5r_sb[:, KC:2 * KC],
                                           rn[:, 1:2], pself[:], OP.mult, OP.add)
            nc.vector.scalar_tensor_tensor(pself[:], w5r_sb[:, 2 * KC:3 * KC],
                                           rn[:, 2:3], pself[:], OP.mult, OP.add)
            kself = wrk.tile([P, KC], f32, tag="kself")
            nc.vector.scalar_tensor_tensor(kself[:], pself[:], NEG_K, pself[:],
                                           OP.mult, OP.max)
            kself_b = wrk.tile([P, KC], bf16, tag="kself_b")
            nc.vector.tensor_scalar_mul(kself_b[:], kself[:], ncl_sb[:, t:t + 1])
            nc.gpsimd.tensor_add(kern[:, 8 * KC:9 * KC],
                                 kern[:, 8 * KC:9 * KC], kself_b[:])

            # ---- tm_k[m] = G[m] * kern2[m, k]; PE shift-accumulate -------
            wide_ps = psw.tile([TS, GW], f32, tag="wide")
            for k in range(K17):
                tm = tpool.tile([P, GW], bf16, tag="tm")
                nc.vector.tensor_mul(
                    tm[:].rearrange("p (o c) -> p o c", c=KC),
                    G[:, GW * t:GW * (t + 1)]
                        .rearrange("p (o c) -> p o c", c=KC),
                    kern[:, KC * k:KC * (k + 1)].unsqueeze(1)
                        .broadcast_to([P, W, KC]))
                nc.tensor.matmul(wide_ps[:, 0:512],
                                 shs_b[:, TS * k:TS * (k + 1)], tm[:, 0:512],
                                 start=(k == 0), stop=(k == K17 - 1),
                                 skip_group_check=True)
                nc.tensor.matmul(wide_ps[:, 512:GW],
                                 shs_b[:, TS * k:TS * (k + 1)], tm[:, 512:GW],
                                 start=(k == 0), stop=(k == K17 - 1),
                                 skip_group_check=True)
            conv = wrk.tile([TS, W], f32, tag="conv")
            nc.vector.tensor_reduce(conv[:],
                                    wide_ps[:].rearrange("p (o c) -> p o c", c=KC),
                                    axis=AX.X, op=OP.add)

            # ---- out = lrelu(conv) @ W_out + x ---------------------------
            ct_ps = ps2.tile([W, TS], f32, tag="ct")
            nc.tensor.matmul(ct_ps[:], conv[:], id_sb[0:TS, 0:TS],
                             is_transpose=True, start=True, stop=True,
                             skip_group_check=True)
            convLT = wrk.tile([W, TS], bf16, tag="convLT")
            nc.scalar.activation(convLT[:], ct_ps[:], AF.Prelu, bias=0.0,
                                 scale=1.0, alpha=a1_sb[0:W, 0:1])
            out_ps = ps2.tile([TS, C], f32, tag="out")
            nc.tensor.matmul(out_ps[:], convLT[:], w_out_b[:],
                             start=True, stop=True, skip_group_check=True)
            out_sb = wrk.tile([TS, C], f32, tag="out_sb")
            nc.vector.tensor_add(out_sb[:], out_ps[:],
                                 xc_all[0:TS, C * t:C * t + C])
            cnt = min(TS, NPC - TS * t)
            nc.sync.dma_start(y[TS * t:TS * t + cnt, :], out_sb[0:cnt, :])

    nc.compile()
    return nc


def _expected_src_dst():
    i = np.arange(N)
    offs = np.arange(-WIN, WIN + 1)
    j = i[:, None] + offs[None, :]
    valid = ((j // L) == (i[:, None] // L)) & (j >= 0) & (j < N)
    j = np.where(valid, j, i[:, None])
    dst = np.repeat(i, offs.size).astype(np.int32)
    src = j.reshape(-1).astype(np.int32)
    return src, dst


def _host_inputs(x, pos, ori, W_in, Ws0, bs0, Wk, W_out):
    xf = np.ascontiguousarray(x.reshape(N, C), np.float32)
    pos = np.asarray(pos, np.float32)
    ori = np.asarray(ori, np.float32)

    WS = np.zeros((136, K17 * KC), np.float32)
    for k in range(K17):
        s = _sidx(k)
        WS[8 * k:8 * k + 7, KC * k:KC * (k + 1)] = Ws0[s]
        WS[8 * k + 7, KC * k:KC * (k + 1)] = bs0[s]
    # wkoc[w, o*KC + c] = Wk[c*W + w, o]
    wkoc = np.ascontiguousarray(
        np.transpose(np.asarray(Wk, np.float32).reshape(KC, W, W),
                     (1, 2, 0)).reshape(W, GW))
    # center gather: nbc[m, k] = po[m + 8 - k]
    shifts_c = np.zeros((128, K17 * 128), np.float32)
    for k in range(K17):
        for m in range(128):
            r = m + 8 - k
            if 0 <= r < 128:
                shifts_c[r, 128 * k + m] = 1.0
    # shift-accumulate: out[n] += tm_k[n + k]
    shifts_s = np.zeros((128, K17 * TS), np.float32)
    for k in range(K17):
        for n in range(TS):
            shifts_s[n + k, TS * k + n] = 1.0
    w5r = np.tile(Ws0[5][3:6].reshape(1, 3 * KC), (128, 1)).astype(np.float32)
    b5r = np.tile(bs0[5].reshape(1, KC), (128, 1)).astype(np.float32)
    common = dict(
        w_in=np.ascontiguousarray(W_in, np.float32),
        ws_a=np.ascontiguousarray(WS[0:128]),
        ws_b=np.ascontiguousarray(WS[128:136]),
        wkoc=wkoc,
        w_out=np.ascontiguousarray(W_out, np.float32),
        ident=np.eye(128, dtype=np.float32),
        shifts_c=shifts_c,
        shifts_s=shifts_s,
        w5r=w5r, b5r=b5r,
        alph1=np.full((128, 1), NEG_IN, np.float32),
        alph2=np.full((128, 1), NEG_K, np.float32),
    )

    offs = np.arange(-WIN, WIN + 1)
    in_maps = []
    for ci in range(NCORES):
        s0 = ci * NPC
        g = s0 - WIN + np.arange(HALO)
        ok = (g >= 0) & (g < N)
        gi = np.clip(g, 0, N - 1)
        x_pad = np.where(ok[:, None], xf[gi], 0.0).astype(np.float32)
        p_pad = np.where(ok[:, None], pos[gi], 0.0).astype(np.float32)
        o_pad = np.where(ok[:, None], ori[gi], 0.0).astype(np.float32)

        xT_slot = np.ascontiguousarray(x_pad.T)                # [128, HALO]

        jj, pp = np.meshgrid(np.arange(NT), np.arange(128), indexing="ij")
        rows = (TS * jj + pp)            # [NT, 128] all < HALO
        po_pad = np.concatenate([p_pad, o_pad], axis=1)  # [HALO, 12]
        po_slot = po_pad[rows].transpose(1, 0, 2).reshape(128, NT * 12)
        rc = WIN + TS * jj + pp
        okc = rc < HALO
        xc_slot = np.where(okc[:, :, None], x_pad[np.minimum(rc, HALO - 1)], 0.0)
        xc_slot = xc_slot.transpose(1, 0, 2).reshape(128, NT * C).astype(np.float32)

        # mask2[m, t, k]: edge (n = m-k, k) exists; ncl2[m, t]: #folded
        # self-loops of node m-8 (kern2/kself live at partition m = n + k)
        mask = np.zeros((128, NT, K17), np.float32)
        ncl = np.zeros((128, NT), np.float32)
        for t in range(NT):
            cnt = min(TS, NPC - TS * t)
            for m in range(128):
                for k in range(K17):
                    n = m - k
                    if 0 <= n < cnt:
                        off = (s0 + TS * t + n) % L
                        if 0 <= off + k - WIN < L:
                            mask[m, t, k] = 1.0
                nn = m - WIN
                if 0 <= nn < cnt:
                    off = (s0 + TS * t + nn) % L
                    v = ((off + offs) >= 0) & ((off + offs) < L)
                    ncl[m, t] = K17 - v.sum()
        in_maps.append(dict(
            xT_slot=xT_slot, xc_slot=xc_slot, po_slot=po_slot,
            maskd=mask.reshape(128, NT * K17), nclmp=ncl, **common))
    return in_maps


def kernel(x, pos, seq, ori, W_in, Ws0, bs0, Wk, W_out, src, dst):
    exp_src, exp_dst = _expected_src_dst()
    assert np.array_equal(np.asarray(src), exp_src), "unexpected src graph"
    assert np.array_equal(np.asarray(dst), exp_dst), "unexpected dst graph"

    from concourse.bass_utils import run_bass_kernel_spmd

    if "nc" not in _PROG:
        _PROG["nc"] = _build_program()
    nc = _PROG["nc"]

    in_maps = _host_inputs(np.asarray(x), np.asarray(pos), np.asarray(ori),
                           np.asarray(W_in), np.asarray(Ws0), np.asarray(bs0),
                           np.asarray(Wk), np.asarray(W_out))
    res = run_bass_kernel_spmd(nc, in_maps, list(range(NCORES)))
    out = np.concatenate([res.results[i]["y"] for i in range(NCORES)], axis=0)
    return out.reshape(B, L, C).astype(np.float32)
